# revision 38
# baseline (speedup 1.0000x reference)
"""Trainium2 Bass kernel for nn_AttachmentPredictor (mask-packed).

Only rows with mask=True contribute to the output (masked exp-norm over
head positions), so the host gathers just those rows (~50%) per core,
batch-major, padded to nblk*512 rows, and the device runs the dense
pipeline on the packed rows:

  stage1: psum[jt] += Wh[dk,jt] @ xT[dk,:]   (fp8 DoubleRow, 4 passes/jt)
          DVE adds the host-computed bf16 per-row bias (prep+child
          projections gathered per packed row) from SBUF into psum;
          the bias costs no PE cycles (v1 used a one-hot E matmul).
          bias_bufs covers the whole group so all bias/xT loads
          prefetch during the previous rep's s2/s3 phases -- shallow
          rings forced ~435GB/s of DMA into the s1 phase window and
          made the kernel bimodal (+3%) under HBM contention.
  tanh -> c1;  stage2: W0 (bf16), tanh -> c2.
  stage3 TRANSPOSED: stationary = c2 row-chunks (data), moving = W1
          k-tiles, psum = [rows, feats]; tanh -> c3T.  The scorer then
          runs on the idle DVE as a free-dim multiply+reduce
          (scalar_tensor_tensor accum_out) -- no PE passes at all.
  scores [128, rc] f32 -> contiguous DMA out (partition-major).

Host: exp(scores), scatter to (b, s), per-batch normalize. The NEFF is
mask-agnostic; only nblk (padded block count) specializes the build.

PE cost model (measured): every matmul pass issues in ~215ns (512
moving cols @ 2.4GHz) regardless of dtype; the moving port carries 16
bits/cycle/partition, so fp8 DR covers 2 k-tiles per pass while bf16
covers 1 -- precision x speed is conserved, and the 2e-2 rel-err gate
(fp8 stage-1 already spends ~1.5e-2 of it) forces stages 2/3 to stay
bf16-grade. Floor: s1 16 + s2 16 + s3T 16 = 48 passes/block; this
kernel measures at that floor with ~0 gap (81.5-83us for 8 blocks;
the last block's s1/s2 windows shrink to the real row count). Measured
dead ends: fp8/split stages 2/3 and half-fp8 stage-3 (error gate or
pass conservation), DMA-to-PSUM bias (unsupported), SBUF bias staging
(slower), partition-strided score DMA (cost 9us), bias DMA issued from
gpsimd (software DGE ~1.3us/issue, 21% regression).
"""

import ml_dtypes
import numpy as np

import concourse.bass as bass
import concourse.mybir as mybir
import concourse.tile as tile
from concourse import bass_utils
from concourse.bass import ts

F32 = mybir.dt.float32
F32R = mybir.dt.float32r
BF16 = mybir.dt.bfloat16
FP8 = mybir.dt.float8e4
AF = mybir.ActivationFunctionType
DR = mybir.MatmulPerfMode.DoubleRow

B, S, D, P = 256, 256, 1024, 512
NCORES = 8
BC = B // NCORES            # 32 batches per core
KD = D // 128               # 8 k-tiles over D
KP = P // 128               # 4 k-tiles over P
EPS = 1e-7

OPTS = {
    "s1_dtype": "fp8",    # stage-1 x/Wh: "f32r" | "bf16" | "fp8" (=> DoubleRow)
    "mm_dtype": "bf16",   # stages 2/3, scorer: "f32r" | "bf16"
    "group": 8,           # blocks per group; stages interleave across the
                          # group so ACT latency hides under sibling matmuls
    "s1_burst": 2,        # stage-1 DR chains emitted for this many blocks
                          # back-to-back (halves DR<->normal mode switches)
    "bias_path": "dve",   # "dve": DVE adds SBUF bias into psum in
                          # place; "dve_sbuf": DVE writes the sum to an
                          # SBUF f32 staging tile (frees the psum slot one
                          # step earlier; ACT reads SBUF — measured ~1%
                          # slower). DMA into PSUM is unsupported.
    "s3_mode": "trans",   # "trans": stage-3 emitted transposed (stationary
                          # = c2 data, moving = W1) so psum/c3 are
                          # [rows, feats] and the scorer runs as a DVE
                          # multiply+reduce along the free dim — no PE
                          # matmuls for the scorer. "normal": PE scorer.
    "sct_dma": "sync",    # engine issuing the scores DMA: "gpsimd" | "sync"
    "xr_bufs": 12,
    "c_bufs": 33,
    "ps_bufs": 4,
    "bias_bufs": 10,
}

_DT = {"f32r": F32R, "bf16": BF16, "f32": F32, "fp8": FP8}
_NPDT = {"f32r": np.float32, "bf16": ml_dtypes.bfloat16, "f32": np.float32,
         "fp8": ml_dtypes.float8_e4m3}


# ---------------------------------------------------------------------------
# walrus in this container accepts at most ONE sync wait per instruction;
# split extra waits onto preceding NoOps on the same engine.
def _split_waits(nc, maxw=1):
    ctr = 0
    for f in nc.m.functions:
        for blk in f.blocks:
            insts = blk.instructions
            newlist = []
            changed = False
            for inst in insts:
                si = inst.sync_info
                if si is not None and len(si.on_wait) > maxw:
                    waits = list(si.on_wait)
                    keep = waits[len(waits) - maxw:]
                    extra = waits[: len(waits) - maxw]
                    for j in range(0, len(extra), maxw):
                        ctr += 1
                        newlist.append(
                            mybir.InstNoOp(
                                name=f"waitsplit-{ctr}",
                                engine=inst.engine,
                                ins=[],
                                outs=[],
                                sync_info=mybir.SyncInfo(
                                    on_wait=extra[j: j + maxw], on_update=[]
                                ),
                            )
                        )
                    inst.sync_info = mybir.SyncInfo(
                        on_wait=keep, on_update=list(si.on_update)
                    )
                    changed = True
                newlist.append(inst)
            if changed:
                insts[:] = newlist


# ---------------------------------------------------------------------------
def _build(opts=None, nblk=9, last_n=512, reps=1, split=True):
    opts = dict(OPTS, **(opts or {}))
    nc = bass.Bass("TRN2", target_bir_lowering=False, debug=False)

    S1DT = _DT[opts["s1_dtype"]]
    MMDT = _DT[opts["mm_dtype"]]
    fp8_s1 = opts["s1_dtype"] == "fp8"

    xT_d = nc.dram_tensor("xT", [nblk, 128, KD * 512], S1DT,
                          kind="ExternalInput").ap()
    bias_d = nc.dram_tensor("biasT", [nblk, 128, KP * 512], BF16,
                            kind="ExternalInput").ap()
    wh_d = nc.dram_tensor("wh", [D, P], S1DT, kind="ExternalInput").ap()
    w0_d = nc.dram_tensor("w0", [P, P], MMDT, kind="ExternalInput").ap()
    w1_d = nc.dram_tensor("w1", [P, P], MMDT, kind="ExternalInput").ap()
    sc_d = nc.dram_tensor("scT", [128, KP], MMDT, kind="ExternalInput").ap()
    scbc_d = nc.dram_tensor("scbc", [128, P], F32, kind="ExternalInput").ap()
    # trans mode: scores stored partition-major [blk, p, rc] so the DMA is
    # contiguous per partition; host reorders (row r = rc*128 + p).
    if opts["s3_mode"] == "trans":
        out_d = nc.dram_tensor("scores", [nblk, 128, KP], F32,
                               kind="ExternalOutput").ap()
    else:
        out_d = nc.dram_tensor("scores", [nblk, 512], F32,
                               kind="ExternalOutput").ap()

    with tile.TileContext(nc) as tc:
        with (
            tc.tile_pool(name="consts", bufs=1) as consts,
            tc.tile_pool(name="ssb", bufs=3) as spool,
            tc.tile_pool(name="xr", bufs=opts["xr_bufs"]) as xpool,
            tc.tile_pool(name="bias", bufs=opts["bias_bufs"]) as bpool,
            tc.tile_pool(name="acts", bufs=opts["c_bufs"]) as cpool,
            tc.tile_pool(name="ps", bufs=opts["ps_bufs"], space="PSUM") as pspool,
        ):
            # ---- constants -------------------------------------------------
            def load_packed(dram, k, n, dt, tag):
                t = consts.tile([128, k * n], dt, tag=tag)
                nc.sync.dma_start(
                    t[:].rearrange("p (k n) -> p k n", n=n),
                    dram.rearrange("(k p) n -> p k n", p=128),
                )
                return t

            wh_r = load_packed(wh_d, KD, P, S1DT, "wh")      # [128, KD*512]
            w0_r = load_packed(w0_d, KP, P, MMDT, "w0")
            w1_r = load_packed(w1_d, KP, P, MMDT, "w1")
            sc_r = consts.tile([128, KP], MMDT, tag="sc")
            nc.sync.dma_start(sc_r[:], sc_d[:])
            scbc_r = consts.tile([128, P], F32, tag="scbc")
            nc.sync.dma_start(scbc_r[:], scbc_d[:])

            # ---- main loop over groups of packed blocks (512 rows each) ----
            G = opts["group"]
            BURST = opts["s1_burst"]
            for _rep in range(reps):
                groups = [list(range(g, min(g + G, nblk)))
                          for g in range(0, nblk, G)]
                def n_of(blk):
                    return last_n if blk == nblk - 1 else 512

                for grp in groups:
                    xrs, biases = {}, {}
                    for blk in grp:
                        n = n_of(blk)
                        xr = xpool.tile([128, KD * 512], S1DT, tag="xr")
                        hw = KD * 512 // 2
                        for h in range(2):
                            nc.sync.dma_start(
                                xr[:, h * hw: (h + 1) * hw],
                                xT_d[blk, :, h * hw: (h + 1) * hw],
                            )
                        xrs[blk] = xr
                        bt = bpool.tile([128, KP * 512], BF16, tag="bias")
                        bw = KP * 512 // 2
                        for h in range(2):
                            # keep bias loads on the sync queue: gpsimd DMA
                            # issue (software DGE, ~1.3us+ init) measured a
                            # 21% regression here
                            nc.sync.dma_start(
                                bt[:, h * bw: (h + 1) * bw],
                                bias_d[blk, :, h * bw: (h + 1) * bw],
                            )
                        biases[blk] = bt

                    # stage 1 (fp8 DR chains; bias added afterwards by DVE
                    # from SBUF so the PE spends no cycles on it). Bursts of
                    # BURST blocks keep the PE in DR mode longer.
                    cs = {blk: [] for blk in grp}
                    whv = wh_r[:].rearrange("p (k n) -> p k n", n=P)
                    bursts = [grp[i:i + BURST] for i in range(0, len(grp), BURST)]
                    for burst in bursts:
                        ps1s = {}
                        for blk in burst:
                            xr, n = xrs[blk], n_of(blk)
                            ps1s[blk] = []
                            xrv = xr[:].rearrange("p (k n) -> p k n", n=512)
                            for jp in range(KP // 2):
                                # merged 2-bank psum pair: halves jt=2jp,2jp+1
                                ps1 = pspool.tile([128, 1024], F32, tag="ps",
                                                  name=f"ps1_{blk}_{jp}")
                                for h in range(2):
                                    jt = 2 * jp + h
                                    for a in range(KD // 2):
                                        nc.tensor.matmul(
                                            ps1[:, h * 512: h * 512 + n],
                                            whv[:, 2 * a: 2 * a + 2,
                                                jt * 128: (jt + 1) * 128],
                                            xrv[:, 2 * a: 2 * a + 2, :n],
                                            start=(a == 0),
                                            stop=(a == KD // 2 - 1),
                                            perf_mode=DR,
                                        )
                                ps1s[blk].append(ps1)
                        for blk in burst:
                            n = n_of(blk)
                            bt = biases[blk]
                            btv = bt[:].rearrange("p (k n) -> p k n", n=512)
                            for jp in range(KP // 2):
                                psv = ps1s[blk][jp][:].rearrange(
                                    "p (k n) -> p k n", n=512)
                                nc.vector.tensor_add(
                                    psv[:, :, :n],
                                    psv[:, :, :n],
                                    btv[:, 2 * jp: 2 * jp + 2, :n],
                                )
                                ct = cpool.tile([128, 1024], MMDT, tag="c1",
                                                bufs=17)
                                ctv = ct[:].rearrange("p (k n) -> p k n",
                                                      n=512)
                                nc.scalar.activation(ctv[:, :, :n],
                                                     psv[:, :, :n], AF.Tanh)
                                cs[blk].append(ct)

                    # stage 2 (block-major within the group)
                    def c1_sl(blk, jk, n):
                        return cs[blk][jk // 2][:, (jk % 2) * 512:
                                                (jk % 2) * 512 + n]
                    for blk in grp:
                        n = n_of(blk)
                        c_out = []
                        for qp in range(KP // 2):
                            ps2 = pspool.tile([128, 1024], F32, tag="ps",
                                              name=f"ps2_{blk}_{qp}")
                            for h in range(2):
                                qt = 2 * qp + h
                                for jk in range(KP):
                                    nc.tensor.matmul(
                                        ps2[:, h * 512: h * 512 + n],
                                        w0_r[:, jk * P + qt * 128:
                                             jk * P + (qt + 1) * 128],
                                        c1_sl(blk, jk, n),
                                        start=(jk == 0),
                                        stop=(jk == KP - 1),
                                    )
                            ct = cpool.tile([128, 1024], MMDT, tag="c2",
                                            bufs=17)
                            psv = ps2[:].rearrange("p (k n) -> p k n", n=512)
                            ctv = ct[:].rearrange("p (k n) -> p k n", n=512)
                            nc.scalar.activation(ctv[:, :, :n], psv[:, :, :n],
                                                 AF.Tanh)
                            c_out.append(ct)
                        cs[blk] = c_out

                    if opts["s3_mode"] == "trans":
                        # stage 3 transposed: stationary = c2 chunks (data),
                        # moving = W1 k-tiles -> psum [row-chunk, feats].
                        # tanh -> c3T; scorer = DVE mult+reduce along free.
                        def c2_sl(blk, jk, lo, sz):
                            return cs[blk][jk // 2][:, (jk % 2) * 512 + lo:
                                                    (jk % 2) * 512 + lo + sz]
                        for blk in grp:
                            n = n_of(blk)
                            nr = (n + 127) // 128
                            sct = spool.tile([128, KP], F32, tag="sct",
                                             bufs=G + 1, name=f"sct_{blk}")
                            for rp in range((nr + 1) // 2):
                                rcs = [rc for rc in (2 * rp, 2 * rp + 1)
                                       if rc < nr]
                                ps3 = pspool.tile([128, 1024], F32, tag="ps",
                                                  name=f"ps3_{blk}_{rp}")
                                mmax = 0
                                for i, rc in enumerate(rcs):
                                    m = min(128, n - rc * 128)
                                    mmax = max(mmax, m)
                                    for jk in range(KP):
                                        nc.tensor.matmul(
                                            ps3[:m, i * 512: (i + 1) * 512],
                                            c2_sl(blk, jk, rc * 128, m),
                                            w1_r[:, jk * P: (jk + 1) * P],
                                            start=(jk == 0),
                                            stop=(jk == KP - 1),
                                        )
                                ct = cpool.tile([128, 1024], MMDT, tag="c3",
                                                bufs=5)
                                w = len(rcs) * 512
                                nc.scalar.activation(ct[:mmax, :w],
                                                     ps3[:mmax, :w], AF.Tanh)
                                for i, rc in enumerate(rcs):
                                    m = min(128, n - rc * 128)
                                    scr = cpool.tile([128, 512], MMDT,
                                                     tag="scscr", bufs=3)
                                    nc.vector.scalar_tensor_tensor(
                                        scr[:m, :],
                                        ct[:m, i * 512: (i + 1) * 512],
                                        1.0,
                                        scbc_r[:m, :],
                                        mybir.AluOpType.mult,
                                        mybir.AluOpType.mult,
                                        accum_out=sct[:m, rc: rc + 1],
                                    )
                            # issue from the idle gpsimd engine to keep the
                            # scores DMA off the busy sync issue queue (DVE
                            # cannot initiate DMAs). gpsimd issue is slow
                            # (software DGE) but this DMA is off-critical.
                            eng = (nc.gpsimd if opts["sct_dma"] == "gpsimd"
                                   else nc.sync)
                            eng.dma_start(out_d[blk, :, :nr],
                                          sct[:, :nr])
                    else:
                        # stage 3 (normal) + PE scorer
                        for blk in grp:
                            n = n_of(blk)
                            c_out = []
                            for qt in range(KP):
                                ps3 = pspool.tile([128, 512], F32, tag="ps",
                                                  name=f"ps3_{blk}_{qt}")
                                for jk in range(KP):
                                    nc.tensor.matmul(
                                        ps3[:, :n],
                                        w1_r[:, jk * P + qt * 128:
                                             jk * P + (qt + 1) * 128],
                                        cs[blk][jk][:, :n],
                                        start=(jk == 0),
                                        stop=(jk == KP - 1),
                                    )
                                ct = cpool.tile([128, 512], MMDT, tag="c3")
                                nc.scalar.activation(ct[:, :n], ps3[:, :n],
                                                     AF.Tanh)
                                c_out.append(ct)
                            cs[blk] = c_out
                            pss = pspool.tile([1, 512], F32, tag="ps",
                                              name=f"pss_{blk}")
                            for qk in range(KP):
                                nc.tensor.matmul(
                                    pss[:, :n],
                                    sc_r[:, qk: qk + 1],
                                    cs[blk][qk][:, :n],
                                    start=(qk == 0),
                                    stop=(qk == KP - 1),
                                )
                            so = spool.tile([1, 512], F32, tag="so",
                                            bufs=G + 1, name=f"so_{blk}")
                            nc.vector.tensor_copy(so[:, :n], pss[:, :n])
                            nc.sync.dma_start(out_d[blk: blk + 1, :n],
                                              so[:, :n])

    if split:
        _split_waits(nc)
    return nc


# ---------------------------------------------------------------------------
def _host_prep(x, proj_head, proj_prep, proj_child, hidden_layers, scorer, mask,
               opts=None):
    opts = dict(OPTS, **(opts or {}))
    s1_np = _NPDT[opts["s1_dtype"]]
    mm_np = _NPDT[opts["mm_dtype"]]
    x = np.asarray(x, np.float32)
    mask = np.asarray(mask)
    wh = np.ascontiguousarray(np.asarray(proj_head, np.float32).astype(s1_np))
    wp = np.asarray(proj_prep, np.float32)
    wc = np.asarray(proj_child, np.float32)
    hl = np.asarray(hidden_layers, np.float32)
    w0 = np.ascontiguousarray(hl[0].astype(mm_np))
    w1 = np.ascontiguousarray(hl[1].astype(mm_np))
    scT = np.ascontiguousarray(
        np.asarray(scorer, np.float32).reshape(KP, 128).T.astype(mm_np)
    )  # [128, 4]
    scbc = np.ascontiguousarray(
        np.broadcast_to(np.asarray(scorer, np.float32)[None, :], (128, P))
    )  # [128, 512] scorer replicated per partition (DVE scorer reduce)

    # Balance row counts across cores: LPT-assign batches (32 per core) so
    # the max per-core packed row count is minimal (usually fits 8 blocks
    # instead of 9 for a ~50% mask). The NEFF is unchanged; the batch
    # permutation is undone in the output scatter via metas.
    counts = mask[:, : S - 2].sum(axis=1)
    order = np.argsort(-counts, kind="stable")
    core_sum = np.zeros(NCORES, np.int64)
    core_n = np.zeros(NCORES, np.int64)
    assign = [[] for _ in range(NCORES)]
    for b in order:
        open_cores = np.nonzero(core_n < BC)[0]
        c = open_cores[np.argmin(core_sum[open_cores])]
        assign[c].append(b)
        core_sum[c] += counts[b]
        core_n[c] += 1
    batches = [np.asarray(a) for a in assign]   # global batch ids per core

    metas = []
    for c in range(NCORES):
        mb = mask[batches[c]][:, : S - 2]
        b_idx, s_idx = np.nonzero(mb)          # batch-major order (local)
        metas.append((batches[c], b_idx, s_idx, len(b_idx)))
    nblk = max(1, max((m[3] + 511) // 512 for m in metas))
    # Ragged last block: shrink only the compute windows (matmul/ACT/DVE);
    # xT/bias DMAs stay dense (padded), avoiding v1's strided-tail-DMA
    # overhead. Round up to a multiple of 8 for alignment.
    maxrows = max(m[3] for m in metas)
    last_n = maxrows - (nblk - 1) * 512
    last_n = min(512, max(8, (last_n + 7) // 8 * 8))

    in_maps = []
    for c in range(NCORES):
        gbatch, b_idx, s_idx, nrows = metas[c]
        xb = x[gbatch]                                      # [32, 256, 1024]
        xP = np.zeros((nblk * 512, D), np.float32)
        xP[:nrows] = xb[b_idx, s_idx]
        if s1_np is ml_dtypes.float8_e4m3:
            np.clip(xP, -240.0, 240.0, out=xP)
        xTc = np.ascontiguousarray(
            xP.reshape(nblk, 512, KD, 128).transpose(0, 3, 2, 1).astype(s1_np)
        ).reshape(nblk, 128, KD * 512)
        # per-row f32 bias (prep+child projections), [nblk, 128, KP*512]
        bias_b = xb[:, S - 2, :] @ wp + xb[:, S - 1, :] @ wc   # [32, 512] f32
        biasP = np.zeros((nblk * 512, P), np.float32)
        biasP[:nrows] = bias_b[b_idx]
        biasT = np.ascontiguousarray(
            biasP.reshape(nblk, 512, KP, 128).transpose(0, 3, 2, 1)
        ).reshape(nblk, 128, KP * 512).astype(ml_dtypes.bfloat16)
        in_maps.append(
            {
                "xT": xTc, "biasT": biasT,
                "wh": wh, "w0": w0, "w1": w1, "scT": scT, "scbc": scbc,
            }
        )
    return in_maps, metas, nblk, last_n


_NC_CACHE = {}


def _get_nc(opts=None, nblk=9, last_n=512):
    opts = dict(OPTS, **(opts or {}))
    key = (opts["s1_dtype"], opts["mm_dtype"], opts["group"],
           opts["s1_burst"], opts["bias_path"], opts["s3_mode"],
           nblk, last_n)
    if key not in _NC_CACHE:
        _NC_CACHE[key] = _build(opts, nblk=nblk, last_n=last_n)
    return _NC_CACHE[key]


def kernel(x, proj_head, proj_prep, proj_child, hidden_layers, scorer, mask):
    in_maps, metas, nblk, last_n = _host_prep(
        x, proj_head, proj_prep, proj_child, hidden_layers, scorer, mask
    )
    nc = _get_nc(nblk=nblk, last_n=last_n)
    res = bass_utils.run_bass_kernel_spmd(
        nc, in_maps, core_ids=list(range(NCORES))
    )
    out = np.zeros((B, S - 2), np.float32)
    for c in range(NCORES):
        gbatch, b_idx, s_idx, nrows = metas[c]
        raw = res.results[c]["scores"]
        if OPTS["s3_mode"] == "trans":
            # [nblk, 128, KP] partition-major -> row-major [nblk*512]
            raw = raw.transpose(0, 2, 1)  # [nblk, KP(rc), 128(p)]
        sc = raw.reshape(-1)[:nrows].astype(np.float64)
        me = np.zeros((BC, S - 2))
        me[b_idx, s_idx] = np.exp(sc)
        sums = me.sum(axis=1, keepdims=True) + EPS
        out[gbatch] = (me / sums).astype(np.float32)
    return out


if __name__ == "__main__":
    rng = np.random.default_rng(0)
    x = rng.standard_normal((B, S, D)).astype(np.float32)
    u = lambda shp: rng.uniform(-0.05, 0.05, shp).astype(np.float32)
    inputs = dict(
        x=x, proj_head=u((D, P)), proj_prep=u((D, P)), proj_child=u((D, P)),
        hidden_layers=u((2, P, P)), scorer=u((P,)),
        mask=rng.integers(0, 2, (B, S)).astype(bool),
    )
    out = kernel(**inputs)
    print("kernel out", out.shape, out.dtype, out[:2, :4])


# revision 39
# speedup vs baseline: 1.0040x; 1.0040x over previous
"""Trainium2 Bass kernel for nn_AttachmentPredictor (mask-packed).

Only rows with mask=True contribute to the output (masked exp-norm over
head positions), so the host gathers just those rows (~50%) per core,
batch-major, padded to nblk*512 rows, and the device runs the dense
pipeline on the packed rows:

  stage1: psum[jt] += Wh[dk,jt] @ xT[dk,:]   (fp8 DoubleRow, 4 passes/jt)
          DVE adds the host-computed bf16 per-row bias (prep+child
          projections gathered per packed row) from SBUF into psum;
          the bias costs no PE cycles (v1 used a one-hot E matmul).
          bias_bufs covers the whole group so all bias/xT loads
          prefetch during the previous rep's s2/s3 phases -- shallow
          rings forced ~435GB/s of DMA into the s1 phase window and
          made the kernel bimodal (+3%) under HBM contention.
  tanh -> c1;  stage2: W0 (bf16), tanh -> c2.
  stage3 TRANSPOSED: stationary = c2 row-chunks (data), moving = W1
          k-tiles, psum = [rows, feats]; tanh -> c3T.  The scorer then
          runs on the idle DVE as a free-dim multiply+reduce
          (scalar_tensor_tensor accum_out) -- no PE passes at all.
  scores [128, rc] f32 -> contiguous DMA out (partition-major).

Host: exp(scores), scatter to (b, s), per-batch normalize. The NEFF is
mask-agnostic; only nblk (padded block count) specializes the build.

PE cost model (measured): every matmul pass issues in ~215ns (512
moving cols @ 2.4GHz) regardless of dtype; the moving port carries 16
bits/cycle/partition, so fp8 DR covers 2 k-tiles per pass while bf16
covers 1 -- precision x speed is conserved, and the 2e-2 rel-err gate
(fp8 stage-1 already spends ~1.5e-2 of it) forces stages 2/3 to stay
bf16-grade. Floor: s1 16 + s2 16 + s3T 16 = 48 passes/block; this
kernel measures at that floor with ~0 gap (~82.5us for 8 blocks;
the last block's s1/s2 windows shrink to the real row count). Psum
tiles are merged [128,1024] 2-bank pairs so ACT drains 2 banks per
instruction (ACT busy 66.6 -> 53us/rep; wall-neutral but adds engine
slack). Residual ~1.3us/rep of 432ns psum-recycle stalls are fixed
sem/ack latency, not engine throughput. Measured
dead ends: fp8/split stages 2/3 and half-fp8 stage-3 (error gate or
pass conservation), DMA-to-PSUM bias (unsupported), SBUF bias staging
(slower), partition-strided score DMA (cost 9us), bias DMA issued from
gpsimd (software DGE ~1.3us/issue, 21% regression).
"""

import ml_dtypes
import numpy as np

import concourse.bass as bass
import concourse.mybir as mybir
import concourse.tile as tile
from concourse import bass_utils
from concourse.bass import ts

F32 = mybir.dt.float32
F32R = mybir.dt.float32r
BF16 = mybir.dt.bfloat16
FP8 = mybir.dt.float8e4
AF = mybir.ActivationFunctionType
DR = mybir.MatmulPerfMode.DoubleRow

B, S, D, P = 256, 256, 1024, 512
NCORES = 8
BC = B // NCORES            # 32 batches per core
KD = D // 128               # 8 k-tiles over D
KP = P // 128               # 4 k-tiles over P
EPS = 1e-7

OPTS = {
    "s1_dtype": "fp8",    # stage-1 x/Wh: "f32r" | "bf16" | "fp8" (=> DoubleRow)
    "mm_dtype": "bf16",   # stages 2/3, scorer: "f32r" | "bf16"
    "group": 8,           # blocks per group; stages interleave across the
                          # group so ACT latency hides under sibling matmuls
    "s1_burst": 2,        # stage-1 DR chains emitted for this many blocks
                          # back-to-back (halves DR<->normal mode switches)
    "bias_path": "dve",   # "dve": DVE adds SBUF bias into psum in
                          # place; "dve_sbuf": DVE writes the sum to an
                          # SBUF f32 staging tile (frees the psum slot one
                          # step earlier; ACT reads SBUF — measured ~1%
                          # slower). DMA into PSUM is unsupported.
    "s3_mode": "trans",   # "trans": stage-3 emitted transposed (stationary
                          # = c2 data, moving = W1) so psum/c3 are
                          # [rows, feats] and the scorer runs as a DVE
                          # multiply+reduce along the free dim — no PE
                          # matmuls for the scorer. "normal": PE scorer.
    "sct_dma": "sync",    # engine issuing the scores DMA: "gpsimd" | "sync"
    "xr_bufs": 12,
    "c_bufs": 33,
    "ps_bufs": 4,
    "bias_bufs": 10,
}

_DT = {"f32r": F32R, "bf16": BF16, "f32": F32, "fp8": FP8}
_NPDT = {"f32r": np.float32, "bf16": ml_dtypes.bfloat16, "f32": np.float32,
         "fp8": ml_dtypes.float8_e4m3}


# ---------------------------------------------------------------------------
# walrus in this container accepts at most ONE sync wait per instruction;
# split extra waits onto preceding NoOps on the same engine.
def _split_waits(nc, maxw=1):
    ctr = 0
    for f in nc.m.functions:
        for blk in f.blocks:
            insts = blk.instructions
            newlist = []
            changed = False
            for inst in insts:
                si = inst.sync_info
                if si is not None and len(si.on_wait) > maxw:
                    waits = list(si.on_wait)
                    keep = waits[len(waits) - maxw:]
                    extra = waits[: len(waits) - maxw]
                    for j in range(0, len(extra), maxw):
                        ctr += 1
                        newlist.append(
                            mybir.InstNoOp(
                                name=f"waitsplit-{ctr}",
                                engine=inst.engine,
                                ins=[],
                                outs=[],
                                sync_info=mybir.SyncInfo(
                                    on_wait=extra[j: j + maxw], on_update=[]
                                ),
                            )
                        )
                    inst.sync_info = mybir.SyncInfo(
                        on_wait=keep, on_update=list(si.on_update)
                    )
                    changed = True
                newlist.append(inst)
            if changed:
                insts[:] = newlist


# ---------------------------------------------------------------------------
def _build(opts=None, nblk=9, last_n=512, reps=1, split=True):
    opts = dict(OPTS, **(opts or {}))
    nc = bass.Bass("TRN2", target_bir_lowering=False, debug=False)

    S1DT = _DT[opts["s1_dtype"]]
    MMDT = _DT[opts["mm_dtype"]]
    fp8_s1 = opts["s1_dtype"] == "fp8"

    xT_d = nc.dram_tensor("xT", [nblk, 128, KD * 512], S1DT,
                          kind="ExternalInput").ap()
    bias_d = nc.dram_tensor("biasT", [nblk, 128, KP * 512], BF16,
                            kind="ExternalInput").ap()
    wh_d = nc.dram_tensor("wh", [D, P], S1DT, kind="ExternalInput").ap()
    w0_d = nc.dram_tensor("w0", [P, P], MMDT, kind="ExternalInput").ap()
    w1_d = nc.dram_tensor("w1", [P, P], MMDT, kind="ExternalInput").ap()
    sc_d = nc.dram_tensor("scT", [128, KP], MMDT, kind="ExternalInput").ap()
    scbc_d = nc.dram_tensor("scbc", [128, P], F32, kind="ExternalInput").ap()
    # trans mode: scores stored partition-major [blk, p, rc] so the DMA is
    # contiguous per partition; host reorders (row r = rc*128 + p).
    if opts["s3_mode"] == "trans":
        out_d = nc.dram_tensor("scores", [nblk, 128, KP], F32,
                               kind="ExternalOutput").ap()
    else:
        out_d = nc.dram_tensor("scores", [nblk, 512], F32,
                               kind="ExternalOutput").ap()

    with tile.TileContext(nc) as tc:
        with (
            tc.tile_pool(name="consts", bufs=1) as consts,
            tc.tile_pool(name="ssb", bufs=3) as spool,
            tc.tile_pool(name="xr", bufs=opts["xr_bufs"]) as xpool,
            tc.tile_pool(name="bias", bufs=opts["bias_bufs"]) as bpool,
            tc.tile_pool(name="acts", bufs=opts["c_bufs"]) as cpool,
            tc.tile_pool(name="ps", bufs=opts["ps_bufs"], space="PSUM") as pspool,
        ):
            # ---- constants -------------------------------------------------
            def load_packed(dram, k, n, dt, tag):
                t = consts.tile([128, k * n], dt, tag=tag)
                nc.sync.dma_start(
                    t[:].rearrange("p (k n) -> p k n", n=n),
                    dram.rearrange("(k p) n -> p k n", p=128),
                )
                return t

            wh_r = load_packed(wh_d, KD, P, S1DT, "wh")      # [128, KD*512]
            w0_r = load_packed(w0_d, KP, P, MMDT, "w0")
            w1_r = load_packed(w1_d, KP, P, MMDT, "w1")
            sc_r = consts.tile([128, KP], MMDT, tag="sc")
            nc.sync.dma_start(sc_r[:], sc_d[:])
            scbc_r = consts.tile([128, P], F32, tag="scbc")
            nc.sync.dma_start(scbc_r[:], scbc_d[:])

            # ---- main loop over groups of packed blocks (512 rows each) ----
            G = opts["group"]
            BURST = opts["s1_burst"]
            for _rep in range(reps):
                groups = [list(range(g, min(g + G, nblk)))
                          for g in range(0, nblk, G)]
                def n_of(blk):
                    return last_n if blk == nblk - 1 else 512

                for grp in groups:
                    xrs, biases = {}, {}
                    for blk in grp:
                        n = n_of(blk)
                        xr = xpool.tile([128, KD * 512], S1DT, tag="xr")
                        hw = KD * 512 // 2
                        for h in range(2):
                            nc.sync.dma_start(
                                xr[:, h * hw: (h + 1) * hw],
                                xT_d[blk, :, h * hw: (h + 1) * hw],
                            )
                        xrs[blk] = xr
                        bt = bpool.tile([128, KP * 512], BF16, tag="bias")
                        bw = KP * 512 // 2
                        for h in range(2):
                            # keep bias loads on the sync queue: gpsimd DMA
                            # issue (software DGE, ~1.3us+ init) measured a
                            # 21% regression here
                            nc.sync.dma_start(
                                bt[:, h * bw: (h + 1) * bw],
                                bias_d[blk, :, h * bw: (h + 1) * bw],
                            )
                        biases[blk] = bt

                    # stage 1 (fp8 DR chains; bias added afterwards by DVE
                    # from SBUF so the PE spends no cycles on it). Bursts of
                    # BURST blocks keep the PE in DR mode longer.
                    cs = {blk: [] for blk in grp}
                    whv = wh_r[:].rearrange("p (k n) -> p k n", n=P)
                    bursts = [grp[i:i + BURST] for i in range(0, len(grp), BURST)]
                    for burst in bursts:
                        ps1s = {}
                        for blk in burst:
                            xr, n = xrs[blk], n_of(blk)
                            ps1s[blk] = []
                            xrv = xr[:].rearrange("p (k n) -> p k n", n=512)
                            for jp in range(KP // 2):
                                # merged 2-bank psum pair: halves jt=2jp,2jp+1
                                ps1 = pspool.tile([128, 1024], F32, tag="ps",
                                                  name=f"ps1_{blk}_{jp}")
                                for h in range(2):
                                    jt = 2 * jp + h
                                    for a in range(KD // 2):
                                        nc.tensor.matmul(
                                            ps1[:, h * 512: h * 512 + n],
                                            whv[:, 2 * a: 2 * a + 2,
                                                jt * 128: (jt + 1) * 128],
                                            xrv[:, 2 * a: 2 * a + 2, :n],
                                            start=(a == 0),
                                            stop=(a == KD // 2 - 1),
                                            perf_mode=DR,
                                        )
                                ps1s[blk].append(ps1)
                        for blk in burst:
                            n = n_of(blk)
                            bt = biases[blk]
                            btv = bt[:].rearrange("p (k n) -> p k n", n=512)
                            for jp in range(KP // 2):
                                psv = ps1s[blk][jp][:].rearrange(
                                    "p (k n) -> p k n", n=512)
                                nc.vector.tensor_add(
                                    psv[:, :, :n],
                                    psv[:, :, :n],
                                    btv[:, 2 * jp: 2 * jp + 2, :n],
                                )
                                ct = cpool.tile([128, 1024], MMDT, tag="c1",
                                                bufs=17)
                                ctv = ct[:].rearrange("p (k n) -> p k n",
                                                      n=512)
                                nc.scalar.activation(ctv[:, :, :n],
                                                     psv[:, :, :n], AF.Tanh)
                                cs[blk].append(ct)

                    # stage 2 (block-major within the group)
                    def c1_sl(blk, jk, n):
                        return cs[blk][jk // 2][:, (jk % 2) * 512:
                                                (jk % 2) * 512 + n]
                    for blk in grp:
                        n = n_of(blk)
                        c_out = []
                        for qp in range(KP // 2):
                            ps2 = pspool.tile([128, 1024], F32, tag="ps",
                                              name=f"ps2_{blk}_{qp}")
                            for h in range(2):
                                qt = 2 * qp + h
                                for jk in range(KP):
                                    nc.tensor.matmul(
                                        ps2[:, h * 512: h * 512 + n],
                                        w0_r[:, jk * P + qt * 128:
                                             jk * P + (qt + 1) * 128],
                                        c1_sl(blk, jk, n),
                                        start=(jk == 0),
                                        stop=(jk == KP - 1),
                                    )
                            ct = cpool.tile([128, 1024], MMDT, tag="c2",
                                            bufs=17)
                            psv = ps2[:].rearrange("p (k n) -> p k n", n=512)
                            ctv = ct[:].rearrange("p (k n) -> p k n", n=512)
                            nc.scalar.activation(ctv[:, :, :n], psv[:, :, :n],
                                                 AF.Tanh)
                            c_out.append(ct)
                        cs[blk] = c_out

                    if opts["s3_mode"] == "trans":
                        # stage 3 transposed: stationary = c2 chunks (data),
                        # moving = W1 k-tiles -> psum [row-chunk, feats].
                        # tanh -> c3T; scorer = DVE mult+reduce along free.
                        def c2_sl(blk, jk, lo, sz):
                            return cs[blk][jk // 2][:, (jk % 2) * 512 + lo:
                                                    (jk % 2) * 512 + lo + sz]
                        for blk in grp:
                            n = n_of(blk)
                            nr = (n + 127) // 128
                            sct = spool.tile([128, KP], F32, tag="sct",
                                             bufs=G + 1, name=f"sct_{blk}")
                            for rp in range((nr + 1) // 2):
                                rcs = [rc for rc in (2 * rp, 2 * rp + 1)
                                       if rc < nr]
                                ps3 = pspool.tile([128, 1024], F32, tag="ps",
                                                  name=f"ps3_{blk}_{rp}")
                                mmax = 0
                                for i, rc in enumerate(rcs):
                                    m = min(128, n - rc * 128)
                                    mmax = max(mmax, m)
                                    for jk in range(KP):
                                        nc.tensor.matmul(
                                            ps3[:m, i * 512: (i + 1) * 512],
                                            c2_sl(blk, jk, rc * 128, m),
                                            w1_r[:, jk * P: (jk + 1) * P],
                                            start=(jk == 0),
                                            stop=(jk == KP - 1),
                                        )
                                ct = cpool.tile([128, 1024], MMDT, tag="c3",
                                                bufs=5)
                                w = len(rcs) * 512
                                nc.scalar.activation(ct[:mmax, :w],
                                                     ps3[:mmax, :w], AF.Tanh)
                                for i, rc in enumerate(rcs):
                                    m = min(128, n - rc * 128)
                                    scr = cpool.tile([128, 512], MMDT,
                                                     tag="scscr", bufs=3)
                                    nc.vector.scalar_tensor_tensor(
                                        scr[:m, :],
                                        ct[:m, i * 512: (i + 1) * 512],
                                        1.0,
                                        scbc_r[:m, :],
                                        mybir.AluOpType.mult,
                                        mybir.AluOpType.mult,
                                        accum_out=sct[:m, rc: rc + 1],
                                    )
                            # issue from the idle gpsimd engine to keep the
                            # scores DMA off the busy sync issue queue (DVE
                            # cannot initiate DMAs). gpsimd issue is slow
                            # (software DGE) but this DMA is off-critical.
                            eng = (nc.gpsimd if opts["sct_dma"] == "gpsimd"
                                   else nc.sync)
                            eng.dma_start(out_d[blk, :, :nr],
                                          sct[:, :nr])
                    else:
                        # stage 3 (normal) + PE scorer
                        for blk in grp:
                            n = n_of(blk)
                            c_out = []
                            for qt in range(KP):
                                ps3 = pspool.tile([128, 512], F32, tag="ps",
                                                  name=f"ps3_{blk}_{qt}")
                                for jk in range(KP):
                                    nc.tensor.matmul(
                                        ps3[:, :n],
                                        w1_r[:, jk * P + qt * 128:
                                             jk * P + (qt + 1) * 128],
                                        cs[blk][jk][:, :n],
                                        start=(jk == 0),
                                        stop=(jk == KP - 1),
                                    )
                                ct = cpool.tile([128, 512], MMDT, tag="c3")
                                nc.scalar.activation(ct[:, :n], ps3[:, :n],
                                                     AF.Tanh)
                                c_out.append(ct)
                            cs[blk] = c_out
                            pss = pspool.tile([1, 512], F32, tag="ps",
                                              name=f"pss_{blk}")
                            for qk in range(KP):
                                nc.tensor.matmul(
                                    pss[:, :n],
                                    sc_r[:, qk: qk + 1],
                                    cs[blk][qk][:, :n],
                                    start=(qk == 0),
                                    stop=(qk == KP - 1),
                                )
                            so = spool.tile([1, 512], F32, tag="so",
                                            bufs=G + 1, name=f"so_{blk}")
                            nc.vector.tensor_copy(so[:, :n], pss[:, :n])
                            nc.sync.dma_start(out_d[blk: blk + 1, :n],
                                              so[:, :n])

    if split:
        _split_waits(nc)
    return nc


# ---------------------------------------------------------------------------
def _host_prep(x, proj_head, proj_prep, proj_child, hidden_layers, scorer, mask,
               opts=None):
    opts = dict(OPTS, **(opts or {}))
    s1_np = _NPDT[opts["s1_dtype"]]
    mm_np = _NPDT[opts["mm_dtype"]]
    x = np.asarray(x, np.float32)
    mask = np.asarray(mask)
    wh = np.ascontiguousarray(np.asarray(proj_head, np.float32).astype(s1_np))
    wp = np.asarray(proj_prep, np.float32)
    wc = np.asarray(proj_child, np.float32)
    hl = np.asarray(hidden_layers, np.float32)
    w0 = np.ascontiguousarray(hl[0].astype(mm_np))
    w1 = np.ascontiguousarray(hl[1].astype(mm_np))
    scT = np.ascontiguousarray(
        np.asarray(scorer, np.float32).reshape(KP, 128).T.astype(mm_np)
    )  # [128, 4]
    scbc = np.ascontiguousarray(
        np.broadcast_to(np.asarray(scorer, np.float32)[None, :], (128, P))
    )  # [128, 512] scorer replicated per partition (DVE scorer reduce)

    # Balance row counts across cores: LPT-assign batches (32 per core) so
    # the max per-core packed row count is minimal (usually fits 8 blocks
    # instead of 9 for a ~50% mask). The NEFF is unchanged; the batch
    # permutation is undone in the output scatter via metas.
    counts = mask[:, : S - 2].sum(axis=1)
    order = np.argsort(-counts, kind="stable")
    core_sum = np.zeros(NCORES, np.int64)
    core_n = np.zeros(NCORES, np.int64)
    assign = [[] for _ in range(NCORES)]
    for b in order:
        open_cores = np.nonzero(core_n < BC)[0]
        c = open_cores[np.argmin(core_sum[open_cores])]
        assign[c].append(b)
        core_sum[c] += counts[b]
        core_n[c] += 1
    batches = [np.asarray(a) for a in assign]   # global batch ids per core

    metas = []
    for c in range(NCORES):
        mb = mask[batches[c]][:, : S - 2]
        b_idx, s_idx = np.nonzero(mb)          # batch-major order (local)
        metas.append((batches[c], b_idx, s_idx, len(b_idx)))
    nblk = max(1, max((m[3] + 511) // 512 for m in metas))
    # Ragged last block: shrink only the compute windows (matmul/ACT/DVE);
    # xT/bias DMAs stay dense (padded), avoiding v1's strided-tail-DMA
    # overhead. Round up to a multiple of 8 for alignment.
    maxrows = max(m[3] for m in metas)
    last_n = maxrows - (nblk - 1) * 512
    last_n = min(512, max(8, (last_n + 7) // 8 * 8))

    in_maps = []
    for c in range(NCORES):
        gbatch, b_idx, s_idx, nrows = metas[c]
        xb = x[gbatch]                                      # [32, 256, 1024]
        xP = np.zeros((nblk * 512, D), np.float32)
        xP[:nrows] = xb[b_idx, s_idx]
        if s1_np is ml_dtypes.float8_e4m3:
            np.clip(xP, -240.0, 240.0, out=xP)
        xTc = np.ascontiguousarray(
            xP.reshape(nblk, 512, KD, 128).transpose(0, 3, 2, 1).astype(s1_np)
        ).reshape(nblk, 128, KD * 512)
        # per-row f32 bias (prep+child projections), [nblk, 128, KP*512]
        bias_b = xb[:, S - 2, :] @ wp + xb[:, S - 1, :] @ wc   # [32, 512] f32
        biasP = np.zeros((nblk * 512, P), np.float32)
        biasP[:nrows] = bias_b[b_idx]
        biasT = np.ascontiguousarray(
            biasP.reshape(nblk, 512, KP, 128).transpose(0, 3, 2, 1)
        ).reshape(nblk, 128, KP * 512).astype(ml_dtypes.bfloat16)
        in_maps.append(
            {
                "xT": xTc, "biasT": biasT,
                "wh": wh, "w0": w0, "w1": w1, "scT": scT, "scbc": scbc,
            }
        )
    return in_maps, metas, nblk, last_n


_NC_CACHE = {}


def _get_nc(opts=None, nblk=9, last_n=512):
    opts = dict(OPTS, **(opts or {}))
    key = (opts["s1_dtype"], opts["mm_dtype"], opts["group"],
           opts["s1_burst"], opts["bias_path"], opts["s3_mode"],
           nblk, last_n)
    if key not in _NC_CACHE:
        _NC_CACHE[key] = _build(opts, nblk=nblk, last_n=last_n)
    return _NC_CACHE[key]


def kernel(x, proj_head, proj_prep, proj_child, hidden_layers, scorer, mask):
    in_maps, metas, nblk, last_n = _host_prep(
        x, proj_head, proj_prep, proj_child, hidden_layers, scorer, mask
    )
    nc = _get_nc(nblk=nblk, last_n=last_n)
    res = bass_utils.run_bass_kernel_spmd(
        nc, in_maps, core_ids=list(range(NCORES))
    )
    out = np.zeros((B, S - 2), np.float32)
    for c in range(NCORES):
        gbatch, b_idx, s_idx, nrows = metas[c]
        raw = res.results[c]["scores"]
        if OPTS["s3_mode"] == "trans":
            # [nblk, 128, KP] partition-major -> row-major [nblk*512]
            raw = raw.transpose(0, 2, 1)  # [nblk, KP(rc), 128(p)]
        sc = raw.reshape(-1)[:nrows].astype(np.float64)
        me = np.zeros((BC, S - 2))
        me[b_idx, s_idx] = np.exp(sc)
        sums = me.sum(axis=1, keepdims=True) + EPS
        out[gbatch] = (me / sums).astype(np.float32)
    return out


if __name__ == "__main__":
    rng = np.random.default_rng(0)
    x = rng.standard_normal((B, S, D)).astype(np.float32)
    u = lambda shp: rng.uniform(-0.05, 0.05, shp).astype(np.float32)
    inputs = dict(
        x=x, proj_head=u((D, P)), proj_prep=u((D, P)), proj_child=u((D, P)),
        hidden_layers=u((2, P, P)), scorer=u((P,)),
        mask=rng.integers(0, 2, (B, S)).astype(bool),
    )
    out = kernel(**inputs)
    print("kernel out", out.shape, out.dtype, out[:2, :4])
